# revision 24
# baseline (speedup 1.0000x reference)
"""EnhancedGNNEncoder Trainium2 kernel: 8-core edge-parallel/node-sharded.

Per layer:  aggr[d] = sum_e w_e*h[src_e] - (sum_e w_e)*h[d] + sum_e b_e
The per-edge scalars (w_e, b_e) depend only on edge_attr/edge_type and the
layer params -- never on h -- so they are precomputed on the host for all L
layers and shipped as one bf16 tensor.  On device each layer is only:
  dma_gather h[src] from a bf16 table -> one-hot windowed matmuls (PSUM
  accumulation) for the weighted segment-sum -> node MLP/LayerNorm/residual
  -> AllGather to rebuild the table for the next layer.
The layer-0 table comes from an on-device AllGather of the fp16 x shard
(instead of uploading a replicated x table); x and the output travel as
fp16 to halve transfer bytes.  Window size = 128 rows (one partition block)
so scatter eviction is a single full-partition PSUM->SBUF copy.
"""
from contextlib import ExitStack

import ml_dtypes
import numpy as np

import concourse.bacc as bacc
import concourse.mybir as mybir
import concourse.tile as tile
from concourse.bass import ds, ts
from concourse.masks import make_identity
from concourse.vector_clock import ScopedClock, VectorClock
from concourse.bass_utils import run_bass_kernel_spmd

F32 = mybir.dt.float32
F16 = mybir.dt.float16
BF16 = mybir.dt.bfloat16
I16 = mybir.dt.int16
I8 = mybir.dt.int8
U8 = mybir.dt.uint8
AF = mybir.ActivationFunctionType
OP = mybir.AluOpType
BF = ml_dtypes.bfloat16

CORES = 8
D = 128          # feature dim (fixed by layout)
W = 128          # nodes per scatter window = one partition block
PUMP = 1
LN_EPS = 1e-5


# ---------------------------------------------------------------------------
# Workaround: this walrus build accepts at most ONE sync-wait per instruction,
# but TileContext._drain_and_barrier attaches every end-of-kernel wait to a
# single Drain.  Emit one single-wait drain per proc instead.
def _patched_drain_and_barrier(self, tick_clock, wait_clock):
    gc = tick_clock.global_clock
    n = len(gc)
    for p in range(n):
        t = gc[p]
        if t <= 0:
            continue
        vec = [0] * n
        vec[p] = t
        d = self.nc.sync.drain()
        wait_clock.add_sem_waits(d.ins, ScopedClock({None: VectorClock(vec)}))
    self.nc.all_engine_barrier()
    popped = self.nc._tile_sem_poison_stack.pop()
    assert popped is self._sem_poison
    self.nc.clear_and_free_semaphores(list(self.sems.allocated().values()))
    self.nc.all_engine_barrier()


tile.TileContext._drain_and_barrier = _patched_drain_and_barrier


def _ceil(a, b):
    return -(-a // b)


# ---------------------------------------------------------------------------
def host_prep(x, edge_attr, node_W, node_b, edge_W, edge_b, emb, ln_g, ln_b,
              fc_W, fc_b, edge_index, node_type, edge_type):
    N = x.shape[0]
    E = edge_attr.shape[0]
    L = node_W.shape[0]
    NT = node_W.shape[1]
    ET = edge_W.shape[1]
    R = N // CORES
    NKC = _ceil(R, 128)
    R_pad = NKC * 128
    NW = NKC                      # windows of 128 rows = partition blocks
    N_tab = R_pad * CORES
    PAGE = N_tab // 2
    assert PAGE < 32768

    src = np.asarray(edge_index[0], np.int64)
    dst = np.asarray(edge_index[1], np.int64)
    e_attr = np.asarray(edge_attr, np.float32)
    e_type = np.asarray(edge_type, np.int64)

    core_of = dst // R
    ld = dst - core_of * R
    win = ld // W
    src_pad = (src // R) * R_pad + (src % R)
    page = src_pad // PAGE

    # per (core, window, page) edge lists
    key = ((core_of * NW + win) * 2 + page).astype(np.int64)
    order = np.argsort(key, kind='stable')
    counts = np.bincount(key[order], minlength=CORES * NW * 2)
    starts = np.zeros(CORES * NW * 2 + 1, np.int64)
    np.cumsum(counts, out=starts[1:])
    counts3 = counts.reshape(CORES, NW, 2)

    # uniform chunk count per (window, page) cell -> fully regular structure
    KCu = int(_ceil(max(int(counts3.max()), 1), 128))
    KC = np.full((NW, 2), KCu, np.int64)
    S0 = NW * KCu * 128
    S1 = S0
    S = S0 + S1
    NCH = S // 128

    meta = dict(N=N, E=E, L=L, NT=NT, ET=ET, R=R, NKC=NKC, R_pad=R_pad,
                NW=NW, N_tab=N_tab, PAGE=PAGE, S0=S0, S1=S1, S=S, NCH=NCH,
                KCu=KCu)

    # ---- per-edge message scalars for every layer (h-independent) ----
    node_W = np.asarray(node_W, np.float32)
    node_b = np.asarray(node_b, np.float32)
    edge_W = np.asarray(edge_W, np.float32)
    edge_b = np.asarray(edge_b, np.float32)
    emb = np.asarray(emb, np.float32)
    ln_g = np.asarray(ln_g, np.float32)
    ln_b = np.asarray(ln_b, np.float32)
    fc_W = np.asarray(fc_W, np.float32)
    fc_b = np.asarray(fc_b, np.float32)

    dirv = e_attr[:, -2]
    pump = e_attr[:, -1]
    spd = pump * np.where(dirv > 0.0, dirv, 1.0)
    sign = dirv * 2.0 - 1.0
    is_pump = (e_type == PUMP)
    Wg = np.empty((L, E), np.float32)
    CB = np.empty((L, 2, N), np.float32)   # C = seg-sum(w), B = seg-sum(b)
    for l in range(L):
        raw = np.empty((E, 2), np.float32)
        for t in range(ET):
            m = e_type == t
            ea = e_attr[m] + emb[l, t]
            raw[m] = ea @ edge_W[l, t].T + edge_b[l, t]
        r0 = raw[:, 0]
        g = np.maximum(r0, 0.0) + np.log1p(np.exp(-np.abs(r0)))
        gain = np.where(is_pump, g * spd, g)
        bias = np.where(is_pump, raw[:, 1] * spd, 0.0)
        Wg[l] = sign * gain
        CB[l, 0] = np.bincount(dst, weights=Wg[l], minlength=N)
        CB[l, 1] = np.bincount(dst, weights=sign * bias, minlength=N)

    per_core = []
    for c in range(CORES):
        slot_src = np.zeros(S, np.int64)
        slot_dcol = np.full(S, float(W), np.float32)
        slot_w = np.zeros((L, S), np.float32)
        s = 0
        for p in range(2):
            for w in range(NW):
                cell = (c * NW + w) * 2 + p
                e0, n_e = starts[cell], counts[cell]
                nslots = int(KC[w, p]) * 128
                el = order[e0:e0 + n_e]
                ne = len(el)
                slot_src[s:s + ne] = src_pad[el] - p * PAGE
                slot_dcol[s:s + ne] = ld[el] - W * w
                slot_w[:, s:s + ne] = Wg[:, el]
                s += nslots
        assert s == S

        idx16 = np.ascontiguousarray(
            slot_src.reshape(-1, 16).T).astype(np.int16)        # [16, S/16]
        dcol = np.ascontiguousarray(
            slot_dcol.reshape(NCH, 128).T.astype(BF))           # [128, NCH]
        wsl = np.ascontiguousarray(
            slot_w.reshape(L, NCH, 128).transpose(0, 2, 1)
            .reshape(L * 128, NCH)).astype(BF)                  # [L*128, NCH]
        cbp = np.zeros((L, 2, R_pad), np.float32)
        cbp[:, :, :R] = CB[:, :, c * R:(c + 1) * R]
        cbp = np.ascontiguousarray(
            cbp.reshape(L * 2, NKC, 128).transpose(0, 2, 1)
            .reshape(L * 2 * 128, NKC))                         # [L*2*128, NKC]

        xs = np.zeros((R_pad, D), np.float16)
        xs[:R] = np.asarray(x[c * R:(c + 1) * R], np.float16)
        nm1 = np.zeros((R_pad,), np.float32)
        nm1[:R] = (np.asarray(node_type[c * R:(c + 1) * R]) == 1)
        nodemask1 = np.ascontiguousarray(
            nm1.reshape(NKC, 128).T.astype(np.int8))

        per_core.append(dict(idx16=idx16, dcol=dcol, w=wsl, cb=cbp,
                             xshard=xs, nodemask1=nodemask1))

    nwT = np.ascontiguousarray(
        node_W.transpose(0, 1, 3, 2)).reshape(L * NT * 128, 128).astype(BF)
    fcwT = np.ascontiguousarray(fc_W.T).astype(BF)
    # broadcast-row vector: node_b | ln_g | ln_b | fc_b  (replicated on device)
    vec = np.concatenate([node_b.reshape(-1), ln_g.reshape(-1),
                          ln_b.reshape(-1), fc_b.reshape(-1)])
    vec = np.ascontiguousarray(vec[None, :]).astype(BF)         # [1, VX]

    # ---- pack everything into one uint8 blob per core (one jax upload) ----
    order_names = ('xshard', 'idx16', 'dcol', 'w', 'cb', 'nodemask1',
                   'nwT', 'fcwT', 'vec')
    shared_arrs = dict(nwT=nwT, fcwT=fcwT, vec=vec)
    offs = {}
    row = 0
    for nm in order_names:
        a = per_core[0][nm] if nm in per_core[0] else shared_arrs[nm]
        nr = _ceil(a.nbytes, 256)
        offs[nm] = (row, nr)
        row += nr
    meta['offs'] = offs
    meta['rows'] = row

    blobs = []
    for c in range(CORES):
        blob = np.zeros((row, 256), np.uint8)
        for nm in order_names:
            a = per_core[c][nm] if nm in per_core[c] else shared_arrs[nm]
            b = np.ascontiguousarray(a).view(np.uint8).reshape(-1)
            r0 = offs[nm][0]
            blob.reshape(-1)[r0 * 256:r0 * 256 + b.size] = b
        blobs.append(dict(blob=blob))

    return blobs, {}, meta


# ---------------------------------------------------------------------------
def build_program(meta, fake_cc=False):
    L, NT = meta['L'], meta['NT']
    NCH, S, S0 = meta['NCH'], meta['S'], meta['S0']
    NKC, R_pad, NW = meta['NKC'], meta['R_pad'], meta['NW']
    N_tab, PAGE = meta['N_tab'], meta['PAGE']
    KCu = meta['KCu']
    VX = L * NT * D + 2 * L * D + D

    nc = bacc.Bacc(trn_type="TRN2", num_devices=CORES)

    offs = meta['offs']
    t_blob = nc.dram_tensor("blob", [meta['rows'], 256], U8,
                            kind="ExternalInput")
    t_out = nc.dram_tensor("out", [R_pad, D], F16, kind="ExternalOutput")

    def sec(name, dt, n):
        r0, nr = offs[name]
        flat = t_blob[r0:r0 + nr, :].bitcast(dt).rearrange("a b -> (a b)")
        return flat[:n]

    agin = [nc.dram_tensor(f"agin{l}", [R_pad, D], BF16) for l in range(L)]
    agout = [nc.dram_tensor(f"agout{l}", [N_tab, D], BF16, addr_space="Shared")
             for l in range(L)]

    def all_gather(l):
        if fake_cc:
            nc.gpsimd.dma_start(out=agout[l][0:R_pad, :], in_=agin[l][:, :])
        else:
            nc.gpsimd.collective_compute(
                "AllGather", OP.bypass,
                replica_groups=[list(range(CORES))],
                ins=[agin[l][:]], outs=[agout[l][:]])

    UN = 1   # loop-body unroll; 1 = smallest program (back-edge ~2us each)

    with tile.TileContext(nc) as tc, ExitStack() as st:
        sb = st.enter_context(tc.tile_pool(name="sb", bufs=1))
        ring2 = st.enter_context(tc.tile_pool(name="ring2", bufs=2))
        ring3 = st.enter_context(tc.tile_pool(name="ring3", bufs=3))
        pT = st.enter_context(tc.tile_pool(name="pT", bufs=1, space="PSUM"))
        pM = st.enter_context(tc.tile_pool(name="pM", bufs=2, space="PSUM"))

        ident = sb.tile([128, 128], F32, name="ident")
        make_identity(nc, ident[:])

        iota = sb.tile([128, 128], BF16, name="iota")
        nc.gpsimd.iota(iota[:, :], [[1, 128]], channel_multiplier=0,
                       allow_small_or_imprecise_dtypes=True)

        # ---- load inputs (carved from the packed blob) ----
        dcolb = sb.tile([128, NCH], BF16, name="dcolb")
        nc.sync.dma_start(
            out=dcolb[:],
            in_=sec('dcol', BF16, 128 * NCH).rearrange("(p q) -> p q", p=128))
        w_sb = sb.tile([128, L * NCH], BF16, name="w_sb")
        nc.sync.dma_start(
            out=w_sb[:].rearrange("p (l q) -> p l q", q=NCH),
            in_=sec('w', BF16, L * 128 * NCH).rearrange(
                "(l p q) -> p l q", p=128, q=NCH))
        cb_sb = sb.tile([128, L * 2 * NKC], F32, name="cb_sb")
        nc.sync.dma_start(
            out=cb_sb[:].rearrange("p (q k) -> p q k", k=NKC),
            in_=sec('cb', F32, L * 2 * 128 * NKC).rearrange(
                "(q p k) -> p q k", p=128, k=NKC))
        idx_src = sec('idx16', I16, S).rearrange("(p q) -> p q", p=16)
        idx_sb = sb.tile([128, S // 16], I16, name="idx_sb")
        for k in range(8):
            nc.sync.dma_start(out=idx_sb[16 * k:16 * k + 16, :], in_=idx_src)
        xh16 = sb.tile([128, NKC * D], F16, name="xh16")
        nc.sync.dma_start(
            out=xh16[:].rearrange("p (k d) -> p k d", d=D),
            in_=sec('xshard', F16, R_pad * D).rearrange(
                "(k p d) -> p k d", p=128, d=D))
        nm1 = sb.tile([128, NKC], I8, name="nm1")
        nc.sync.dma_start(
            out=nm1[:],
            in_=sec('nodemask1', I8, 128 * NKC).rearrange(
                "(p k) -> p k", p=128))
        nwT_sb = sb.tile([128, L * NT * D], BF16, name="nwT_sb")
        nc.sync.dma_start(
            out=nwT_sb[:].rearrange("p (l d) -> p l d", d=D),
            in_=sec('nwT', BF16, L * NT * 128 * D).rearrange(
                "(l p d) -> p l d", p=128, d=D))
        fcw_sb = sb.tile([128, D], BF16, name="fcw_sb")
        nc.sync.dma_start(
            out=fcw_sb[:],
            in_=sec('fcwT', BF16, 128 * D).rearrange("(p d) -> p d", p=128))
        vec_sb = sb.tile([1, VX], BF16, name="vec_sb")
        nc.sync.dma_start(
            out=vec_sb[:],
            in_=sec('vec', BF16, VX).rearrange("(p q) -> p q", p=1))

        # ---- broadcast vec across partitions via K=1 matmul ----
        ones1 = sb.tile([1, 128], BF16, name="ones1")
        nc.vector.memset(ones1[:], 1.0)
        bcast = sb.tile([128, VX], F32, name="bcast")
        nv = _ceil(VX, 512)
        for i in range(nv):
            cw = min(512, VX - i * 512)
            pb = pT.tile([128, 512], F32, name="pb", tag="pb")
            nc.tensor.matmul(out=pb[:, :cw], lhsT=ones1[:, :],
                             rhs=vec_sb[:, i * 512:i * 512 + cw],
                             start=True, stop=True)
            nc.vector.tensor_copy(out=bcast[:, i * 512:i * 512 + cw],
                                  in_=pb[:, :cw])
        nbr = bcast[:, 0:L * NT * D]
        grp = bcast[:, L * NT * D:L * NT * D + L * D]
        brp = bcast[:, L * NT * D + L * D:L * NT * D + 2 * L * D]
        fcb = bcast[:, L * NT * D + 2 * L * D:VX]

        epsc = sb.tile([128, 1], F32, name="epsc")
        nc.vector.memset(epsc[:], LN_EPS)

        # ---- h init + layer-0 gather table via AllGather(x) ----
        h_sb = sb.tile([128, NKC * D], F32, name="h_sb")
        nc.vector.tensor_copy(out=h_sb[:], in_=xh16[:])
        nc.gpsimd.dma_start(
            out=agin[0][:].rearrange("(k p) d -> p k d", p=128),
            in_=xh16[:].rearrange("p (k d) -> p k d", d=D))
        all_gather(0)

        aggr_sb = sb.tile([128, NKC * D], F32, name="aggr_sb")

        for l in range(L):
            w_l = w_sb[:, l * NCH:(l + 1) * NCH]
            C_l = cb_sb[:, (2 * l) * NKC:(2 * l + 1) * NKC]
            B_l = cb_sb[:, (2 * l + 1) * NKC:(2 * l + 2) * NKC]
            table = agout[l]

            # ------- gather + scatter (hw loop over windows, per pass) -----
            def cell_body(p, wv):
                # dynamic chunk offset for this (window, page) cell
                coff = ds(p * NW * KCu + wv * KCu, KCu)
                hsrc = ring3.tile([128, KCu * D], BF16, name="hsrc",
                                  tag="hsrc")
                nc.gpsimd.dma_gather(
                    out_ap=hsrc[:, :].rearrange("p (n d) -> p n d", d=D),
                    in_ap=table[p * PAGE:(p + 1) * PAGE, :],
                    idxs_ap=idx_sb[:, ds(p * NW * KCu * 8 + wv * (KCu * 8),
                                         KCu * 8)],
                    num_idxs=KCu * 128,
                    num_idxs_reg=KCu * 128,
                    elem_size=D,
                    single_packet=False)
                eqr = ring3.tile([128, KCu * 128], BF16, name="eqr",
                                 tag="eqr")
                eqv = eqr[:, :].rearrange("p (c t) -> p c t", t=128)
                nc.vector.tensor_tensor(
                    out=eqv,
                    in0=dcolb[:, coff, None].to_broadcast([128, KCu, 128]),
                    in1=iota[:, None, :].to_broadcast([128, KCu, 128]),
                    op=OP.is_equal)
                # scale one-hot by w_e in place (exact: rows are 0/1)
                nc.vector.tensor_tensor(
                    out=eqv, in0=eqv,
                    in1=w_l[:, coff][:, :, None].to_broadcast(
                        [128, KCu, 128]),
                    op=OP.mult)
                pmw = pM.tile([128, D], F32, name="pmw", tag="pmain",
                              bufs=2)
                for ci in range(KCu):
                    nc.tensor.matmul(
                        out=pmw[:, :],
                        lhsT=eqr[:, ci * 128:ci * 128 + 128],
                        rhs=hsrc[:, ci * D:(ci + 1) * D],
                        start=ci == 0, stop=ci == KCu - 1,
                        skip_group_check=True)
                ws = ts(wv, D)
                if p == 0:
                    nc.vector.tensor_copy(out=aggr_sb[:, ws], in_=pmw[:, :])
                else:
                    tcorr = ring3.tile([128, D], F32, name="tcorr",
                                       tag="tcorr")
                    tmul = ring3.tile([128, D], F32, name="tmul", tag="tmul")
                    nc.vector.tensor_tensor(
                        out=tcorr[:, :], in0=pmw[:, :],
                        in1=aggr_sb[:, ws], op=OP.add)
                    nc.vector.tensor_scalar(
                        tmul[:, :], h_sb[:, ws], C_l[:, ds(wv, 1)],
                        B_l[:, ds(wv, 1)], OP.mult, OP.subtract)
                    nc.vector.tensor_tensor(
                        out=aggr_sb[:, ws], in0=tcorr[:, :],
                        in1=tmul[:, :], op=OP.subtract)

            for p in range(2):
                with tc.For_i(0, NW, UN) as wb:
                    for u in range(UN):
                        cell_body(p, wb + u)

            # ------------- node phase (hw loop, 7x unrolled) -------------
            def node_body(kv):
                ks = ts(kv, D)
                astage = ring2.tile([128, D], BF16, name="astage",
                                    tag="astage")
                nc.vector.tensor_copy(out=astage[:, :], in_=aggr_sb[:, ks])
                aggT = ring2.tile([128, D], BF16, name="aggT", tag="aggT")
                nc.sync.dma_start_transpose(aggT[:, :], astage[:, :])
                pmlp = pM.tile([128, 2 * D], F32, name="pmlp", tag="pmlp",
                               bufs=2)
                for t in range(NT):
                    nwv = nwT_sb[:, (l * NT + t) * D:(l * NT + t + 1) * D]
                    nc.tensor.matmul(out=pmlp[:, t * D:(t + 1) * D],
                                     lhsT=aggT[:, :], rhs=nwv,
                                     start=True, stop=True,
                                     skip_group_check=True)
                ssel = ring3.tile([128, D], F32, name="ssel", tag="ssel")
                stmp = ring3.tile([128, D], F32, name="stmp", tag="stmp")
                nc.vector.tensor_tensor(
                    out=ssel[:, :], in0=pmlp[:, 0:D],
                    in1=nbr[:, (l * NT) * D:(l * NT + 1) * D], op=OP.add)
                nc.vector.tensor_tensor(
                    out=stmp[:, :], in0=pmlp[:, D:2 * D],
                    in1=nbr[:, (l * NT + 1) * D:(l * NT + 2) * D], op=OP.add)
                nc.vector.copy_predicated(
                    ssel[:, :], nm1[:, ds(kv, 1)].to_broadcast([128, D]),
                    stmp[:, :])
                hrelu = ring3.tile([128, D], F32, name="hrelu", tag="hrelu")
                sqscr = ring3.tile([128, D], F32, name="sqscr", tag="sqscr")
                musum = ring3.tile([128, 4], F32, name="musum", tag="musum")
                nc.scalar.activation(hrelu[:, :], ssel[:, :], AF.Relu,
                                     accum_out=musum[:, 0:1])
                nc.vector.tensor_scalar_mul(musum[:, 1:2], musum[:, 0:1],
                                            -1.0 / D)
                nc.scalar.activation(sqscr[:, :], hrelu[:, :], AF.Square,
                                     bias=musum[:, 1:2], scale=1.0,
                                     accum_out=musum[:, 2:3])
                nc.scalar.activation(musum[:, 3:4], musum[:, 2:3], AF.Sqrt,
                                     bias=epsc[:, 0:1], scale=1.0 / D)
                rstd = ring3.tile([128, 1], F32, name="rstd", tag="rstd")
                nc.vector.reciprocal(rstd[:, :], musum[:, 3:4])
                nc.vector.tensor_scalar(
                    stmp[:, :], hrelu[:, :], musum[:, 1:2], rstd[:, 0:1],
                    OP.add, OP.mult)
                nc.vector.tensor_tensor(
                    out=stmp[:, :], in0=stmp[:, :],
                    in1=grp[:, l * D:(l + 1) * D], op=OP.mult)
                nc.vector.tensor_tensor(
                    out=stmp[:, :], in0=stmp[:, :],
                    in1=brp[:, l * D:(l + 1) * D], op=OP.add)
                nc.vector.tensor_tensor(
                    out=h_sb[:, ks], in0=stmp[:, :], in1=h_sb[:, ks],
                    op=OP.add)

            with tc.For_i(0, NKC, UN) as kbase:
                for u in range(UN):
                    node_body(kbase + u)

            if l < L - 1:
                nc.gpsimd.dma_start(
                    out=agin[l + 1][:].rearrange("(k p) d -> p k d", p=128),
                    in_=h_sb[:].rearrange("p (k d) -> p k d", d=D))
                all_gather(l + 1)

        # ------------- final fc (hw loop, 7x unrolled) -------------
        def fc_body(kv):
            ks = ts(kv, D)
            hstage = ring2.tile([128, D], BF16, name="hstage", tag="astage")
            nc.vector.tensor_copy(out=hstage[:, :], in_=h_sb[:, ks])
            hT = ring2.tile([128, D], BF16, name="hT", tag="aggT")
            nc.sync.dma_start_transpose(hT[:, :], hstage[:, :])
            pfc = pM.tile([128, D], F32, name="pfc", tag="pmlp", bufs=2)
            nc.tensor.matmul(out=pfc[:, :], lhsT=hT[:, :], rhs=fcw_sb[:, :],
                             start=True, stop=True, skip_group_check=True)
            osb = ring2.tile([128, D], F16, name="osb", tag="osb")
            nc.vector.tensor_tensor(out=osb[:, :], in0=pfc[:, :],
                                    in1=fcb[:, :], op=OP.add)
            nc.sync.dma_start(out=t_out[ts(kv, 128), :], in_=osb[:, :])

        with tc.For_i(0, NKC, UN) as kbase:
            for u in range(UN):
                fc_body(kbase + u)

    nc.compile()
    return nc


# ---------------------------------------------------------------------------
_CACHE = {}


def kernel(**inputs):
    per_core, shared, meta = host_prep(**inputs)
    key = (meta['S'], meta['S0'], meta['S1'], meta['N'], meta['L'],
           meta['KCu'])
    if key not in _CACHE:
        _CACHE[key] = build_program(meta)
    nc = _CACHE[key]

    in_maps = []
    for c in range(CORES):
        pc = per_core[c]
        in_maps.append(dict(blob=per_core[c]['blob']))

    import os
    import time as _time
    trace = os.environ.get("KTRACE", "0") == "1"
    _t0 = _time.time()
    res = run_bass_kernel_spmd(nc, in_maps, core_ids=list(range(CORES)),
                               trace=trace)
    kernel.last_exec_wall = _time.time() - _t0
    R = meta['R']
    out = np.concatenate(
        [res.results[c]["out"][:R] for c in range(CORES)], axis=0)
    kernel.last_results = res
    return out.astype(np.float32)


# revision 25
# speedup vs baseline: 1.0725x; 1.0725x over previous
"""EnhancedGNNEncoder Trainium2 kernel: 8-core edge-parallel/node-sharded.

Per layer:  aggr[d] = sum_e w_e*h[src_e] - (sum_e w_e)*h[d] + sum_e b_e
The per-edge scalars (w_e, b_e) depend only on edge_attr/edge_type and the
layer params -- never on h -- so they are precomputed on the host for all L
layers and shipped as one bf16 tensor.  On device each layer is only:
  dma_gather h[src] from a bf16 table -> one-hot windowed matmuls (PSUM
  accumulation) for the weighted segment-sum -> node MLP/LayerNorm/residual
  -> AllGather to rebuild the table for the next layer.
The layer-0 table comes from an on-device AllGather of the fp16 x shard
(instead of uploading a replicated x table); x and the output travel as
fp16 to halve transfer bytes.  Window size = 128 rows (one partition block)
so scatter eviction is a single full-partition PSUM->SBUF copy.
"""
from contextlib import ExitStack

import ml_dtypes
import numpy as np

import concourse.bacc as bacc
import concourse.mybir as mybir
import concourse.tile as tile
from concourse.bass import ds, ts
from concourse.masks import make_identity
from concourse.vector_clock import ScopedClock, VectorClock
from concourse.bass_utils import run_bass_kernel_spmd

F32 = mybir.dt.float32
F16 = mybir.dt.float16
BF16 = mybir.dt.bfloat16
I16 = mybir.dt.int16
I8 = mybir.dt.int8
U8 = mybir.dt.uint8
AF = mybir.ActivationFunctionType
OP = mybir.AluOpType
BF = ml_dtypes.bfloat16

CORES = 8
D = 128          # feature dim (fixed by layout)
W = 128          # nodes per scatter window = one partition block
PUMP = 1
LN_EPS = 1e-5


# ---------------------------------------------------------------------------
# Workaround: this walrus build accepts at most ONE sync-wait per instruction,
# but TileContext._drain_and_barrier attaches every end-of-kernel wait to a
# single Drain.  Emit one single-wait drain per proc instead.
def _patched_drain_and_barrier(self, tick_clock, wait_clock):
    gc = tick_clock.global_clock
    n = len(gc)
    for p in range(n):
        t = gc[p]
        if t <= 0:
            continue
        vec = [0] * n
        vec[p] = t
        d = self.nc.sync.drain()
        wait_clock.add_sem_waits(d.ins, ScopedClock({None: VectorClock(vec)}))
    self.nc.all_engine_barrier()
    popped = self.nc._tile_sem_poison_stack.pop()
    assert popped is self._sem_poison
    self.nc.clear_and_free_semaphores(list(self.sems.allocated().values()))
    self.nc.all_engine_barrier()


tile.TileContext._drain_and_barrier = _patched_drain_and_barrier


def _ceil(a, b):
    return -(-a // b)


# ---------------------------------------------------------------------------
def host_prep(x, edge_attr, node_W, node_b, edge_W, edge_b, emb, ln_g, ln_b,
              fc_W, fc_b, edge_index, node_type, edge_type):
    N = x.shape[0]
    E = edge_attr.shape[0]
    L = node_W.shape[0]
    NT = node_W.shape[1]
    ET = edge_W.shape[1]
    R = N // CORES
    NKC = _ceil(R, 128)
    R_pad = NKC * 128
    NW = NKC                      # windows of 128 rows = partition blocks
    N_tab = R_pad * CORES
    PAGE = N_tab // 2
    assert PAGE < 32768

    src = np.asarray(edge_index[0], np.int64)
    dst = np.asarray(edge_index[1], np.int64)
    e_attr = np.asarray(edge_attr, np.float32)
    e_type = np.asarray(edge_type, np.int64)

    core_of = dst // R
    ld = dst - core_of * R
    win = ld // W
    src_pad = (src // R) * R_pad + (src % R)
    page = src_pad // PAGE

    # per (core, window, page) edge lists
    key = ((core_of * NW + win) * 2 + page).astype(np.int64)
    order = np.argsort(key, kind='stable')
    counts = np.bincount(key[order], minlength=CORES * NW * 2)
    starts = np.zeros(CORES * NW * 2 + 1, np.int64)
    np.cumsum(counts, out=starts[1:])
    counts3 = counts.reshape(CORES, NW, 2)

    # uniform chunk count per (window, page) cell -> fully regular structure
    KCu = int(_ceil(max(int(counts3.max()), 1), 128))
    KC = np.full((NW, 2), KCu, np.int64)
    S0 = NW * KCu * 128
    S1 = S0
    S = S0 + S1
    NCH = S // 128

    meta = dict(N=N, E=E, L=L, NT=NT, ET=ET, R=R, NKC=NKC, R_pad=R_pad,
                NW=NW, N_tab=N_tab, PAGE=PAGE, S0=S0, S1=S1, S=S, NCH=NCH,
                KCu=KCu)

    # ---- per-edge message scalars for every layer (h-independent) ----
    node_W = np.asarray(node_W, np.float32)
    node_b = np.asarray(node_b, np.float32)
    edge_W = np.asarray(edge_W, np.float32)
    edge_b = np.asarray(edge_b, np.float32)
    emb = np.asarray(emb, np.float32)
    ln_g = np.asarray(ln_g, np.float32)
    ln_b = np.asarray(ln_b, np.float32)
    fc_W = np.asarray(fc_W, np.float32)
    fc_b = np.asarray(fc_b, np.float32)

    dirv = e_attr[:, -2]
    pump = e_attr[:, -1]
    spd = pump * np.where(dirv > 0.0, dirv, 1.0)
    sign = dirv * 2.0 - 1.0
    is_pump = (e_type == PUMP)
    Wg = np.empty((L, E), np.float32)
    CB = np.empty((L, 2, N), np.float32)   # C = seg-sum(w), B = seg-sum(b)
    for l in range(L):
        raw = np.empty((E, 2), np.float32)
        for t in range(ET):
            m = e_type == t
            ea = e_attr[m] + emb[l, t]
            raw[m] = ea @ edge_W[l, t].T + edge_b[l, t]
        r0 = raw[:, 0]
        g = np.maximum(r0, 0.0) + np.log1p(np.exp(-np.abs(r0)))
        gain = np.where(is_pump, g * spd, g)
        bias = np.where(is_pump, raw[:, 1] * spd, 0.0)
        Wg[l] = sign * gain
        CB[l, 0] = np.bincount(dst, weights=Wg[l], minlength=N)
        CB[l, 1] = np.bincount(dst, weights=sign * bias, minlength=N)

    per_core = []
    for c in range(CORES):
        slot_src = np.zeros(S, np.int64)
        slot_dcol = np.full(S, float(W), np.float32)
        slot_w = np.zeros((L, S), np.float32)
        s = 0
        for p in range(2):
            for w in range(NW):
                cell = (c * NW + w) * 2 + p
                e0, n_e = starts[cell], counts[cell]
                nslots = int(KC[w, p]) * 128
                el = order[e0:e0 + n_e]
                ne = len(el)
                slot_src[s:s + ne] = src_pad[el] - p * PAGE
                slot_dcol[s:s + ne] = ld[el] - W * w
                slot_w[:, s:s + ne] = Wg[:, el]
                s += nslots
        assert s == S

        idx16 = np.ascontiguousarray(
            slot_src.reshape(-1, 16).T).astype(np.int16)        # [16, S/16]
        dcol = np.ascontiguousarray(
            slot_dcol.reshape(NCH, 128).T.astype(BF))           # [128, NCH]
        wsl = np.ascontiguousarray(
            slot_w.reshape(L, NCH, 128).transpose(0, 2, 1)
            .reshape(L * 128, NCH)).astype(BF)                  # [L*128, NCH]
        cbp = np.zeros((L, 2, R_pad), np.float32)
        cbp[:, :, :R] = CB[:, :, c * R:(c + 1) * R]
        cbp = np.ascontiguousarray(
            cbp.reshape(L * 2, NKC, 128).transpose(0, 2, 1)
            .reshape(L * 2 * 128, NKC))                         # [L*2*128, NKC]

        xs = np.zeros((R_pad, D), np.float16)
        xs[:R] = np.asarray(x[c * R:(c + 1) * R], np.float16)
        nm1 = np.zeros((R_pad,), np.float32)
        nm1[:R] = (np.asarray(node_type[c * R:(c + 1) * R]) == 1)
        nodemask1 = np.ascontiguousarray(
            nm1.reshape(NKC, 128).T.astype(np.int8))

        per_core.append(dict(idx16=idx16, dcol=dcol, w=wsl, cb=cbp,
                             xshard=xs, nodemask1=nodemask1))

    nwT = np.ascontiguousarray(
        node_W.transpose(0, 1, 3, 2)).reshape(L * NT * 128, 128).astype(BF)
    fcwT = np.ascontiguousarray(fc_W.T).astype(BF)
    # broadcast-row vector: node_b | ln_g | ln_b | fc_b  (replicated on device)
    vec = np.concatenate([node_b.reshape(-1), ln_g.reshape(-1),
                          ln_b.reshape(-1), fc_b.reshape(-1)])
    vec = np.ascontiguousarray(vec[None, :]).astype(BF)         # [1, VX]

    # ---- pack everything into one uint8 blob per core (one jax upload) ----
    order_names = ('xshard', 'idx16', 'dcol', 'w', 'cb', 'nodemask1',
                   'nwT', 'fcwT', 'vec')
    shared_arrs = dict(nwT=nwT, fcwT=fcwT, vec=vec)
    offs = {}
    row = 0
    for nm in order_names:
        a = per_core[0][nm] if nm in per_core[0] else shared_arrs[nm]
        nr = _ceil(a.nbytes, 256)
        offs[nm] = (row, nr)
        row += nr
    meta['offs'] = offs
    meta['rows'] = row

    blobs = []
    for c in range(CORES):
        blob = np.zeros((row, 256), np.uint8)
        for nm in order_names:
            a = per_core[c][nm] if nm in per_core[c] else shared_arrs[nm]
            b = np.ascontiguousarray(a).view(np.uint8).reshape(-1)
            r0 = offs[nm][0]
            blob.reshape(-1)[r0 * 256:r0 * 256 + b.size] = b
        blobs.append(dict(blob=blob))

    return blobs, {}, meta


# ---------------------------------------------------------------------------
def build_program(meta, fake_cc=False):
    L, NT = meta['L'], meta['NT']
    NCH, S, S0 = meta['NCH'], meta['S'], meta['S0']
    NKC, R_pad, NW = meta['NKC'], meta['R_pad'], meta['NW']
    N_tab, PAGE = meta['N_tab'], meta['PAGE']
    KCu = meta['KCu']
    VX = L * NT * D + 2 * L * D + D

    nc = bacc.Bacc(trn_type="TRN2", num_devices=CORES)

    offs = meta['offs']
    t_blob = nc.dram_tensor("blob", [meta['rows'], 256], U8,
                            kind="ExternalInput")
    t_out = nc.dram_tensor("out", [R_pad, D], F16, kind="ExternalOutput")

    def sec(name, dt, n):
        r0, nr = offs[name]
        flat = t_blob[r0:r0 + nr, :].bitcast(dt).rearrange("a b -> (a b)")
        return flat[:n]

    agin = [nc.dram_tensor(f"agin{l}", [R_pad, D], BF16) for l in range(L)]
    agout = [nc.dram_tensor(f"agout{l}", [N_tab, D], BF16, addr_space="Shared")
             for l in range(L)]

    def all_gather(l):
        if fake_cc:
            nc.gpsimd.dma_start(out=agout[l][0:R_pad, :], in_=agin[l][:, :])
        else:
            nc.gpsimd.collective_compute(
                "AllGather", OP.bypass,
                replica_groups=[list(range(CORES))],
                ins=[agin[l][:]], outs=[agout[l][:]])

    UN = max(d for d in range(1, 9) if NKC % d == 0)   # loop-body unroll

    with tile.TileContext(nc) as tc, ExitStack() as st:
        sb = st.enter_context(tc.tile_pool(name="sb", bufs=1))
        ring2 = st.enter_context(tc.tile_pool(name="ring2", bufs=2))
        ring3 = st.enter_context(tc.tile_pool(name="ring3", bufs=3))
        pT = st.enter_context(tc.tile_pool(name="pT", bufs=1, space="PSUM"))
        pM = st.enter_context(tc.tile_pool(name="pM", bufs=2, space="PSUM"))

        ident = sb.tile([128, 128], F32, name="ident")
        make_identity(nc, ident[:])

        iota = sb.tile([128, 128], BF16, name="iota")
        nc.gpsimd.iota(iota[:, :], [[1, 128]], channel_multiplier=0,
                       allow_small_or_imprecise_dtypes=True)

        # ---- load inputs (carved from the packed blob) ----
        dcolb = sb.tile([128, NCH], BF16, name="dcolb")
        nc.sync.dma_start(
            out=dcolb[:],
            in_=sec('dcol', BF16, 128 * NCH).rearrange("(p q) -> p q", p=128))
        w_sb = sb.tile([128, L * NCH], BF16, name="w_sb")
        nc.sync.dma_start(
            out=w_sb[:].rearrange("p (l q) -> p l q", q=NCH),
            in_=sec('w', BF16, L * 128 * NCH).rearrange(
                "(l p q) -> p l q", p=128, q=NCH))
        cb_sb = sb.tile([128, L * 2 * NKC], F32, name="cb_sb")
        nc.sync.dma_start(
            out=cb_sb[:].rearrange("p (q k) -> p q k", k=NKC),
            in_=sec('cb', F32, L * 2 * 128 * NKC).rearrange(
                "(q p k) -> p q k", p=128, k=NKC))
        idx_src = sec('idx16', I16, S).rearrange("(p q) -> p q", p=16)
        idx_sb = sb.tile([128, S // 16], I16, name="idx_sb")
        for k in range(8):
            nc.sync.dma_start(out=idx_sb[16 * k:16 * k + 16, :], in_=idx_src)
        xh16 = sb.tile([128, NKC * D], F16, name="xh16")
        nc.sync.dma_start(
            out=xh16[:].rearrange("p (k d) -> p k d", d=D),
            in_=sec('xshard', F16, R_pad * D).rearrange(
                "(k p d) -> p k d", p=128, d=D))
        nm1 = sb.tile([128, NKC], I8, name="nm1")
        nc.sync.dma_start(
            out=nm1[:],
            in_=sec('nodemask1', I8, 128 * NKC).rearrange(
                "(p k) -> p k", p=128))
        nwT_sb = sb.tile([128, L * NT * D], BF16, name="nwT_sb")
        nc.sync.dma_start(
            out=nwT_sb[:].rearrange("p (l d) -> p l d", d=D),
            in_=sec('nwT', BF16, L * NT * 128 * D).rearrange(
                "(l p d) -> p l d", p=128, d=D))
        fcw_sb = sb.tile([128, D], BF16, name="fcw_sb")
        nc.sync.dma_start(
            out=fcw_sb[:],
            in_=sec('fcwT', BF16, 128 * D).rearrange("(p d) -> p d", p=128))
        vec_sb = sb.tile([1, VX], BF16, name="vec_sb")
        nc.sync.dma_start(
            out=vec_sb[:],
            in_=sec('vec', BF16, VX).rearrange("(p q) -> p q", p=1))

        # ---- broadcast vec across partitions via K=1 matmul ----
        ones1 = sb.tile([1, 128], BF16, name="ones1")
        nc.vector.memset(ones1[:], 1.0)
        bcast = sb.tile([128, VX], F32, name="bcast")
        nv = _ceil(VX, 512)
        for i in range(nv):
            cw = min(512, VX - i * 512)
            pb = pT.tile([128, 512], F32, name="pb", tag="pb")
            nc.tensor.matmul(out=pb[:, :cw], lhsT=ones1[:, :],
                             rhs=vec_sb[:, i * 512:i * 512 + cw],
                             start=True, stop=True)
            nc.vector.tensor_copy(out=bcast[:, i * 512:i * 512 + cw],
                                  in_=pb[:, :cw])
        nbr = bcast[:, 0:L * NT * D]
        grp = bcast[:, L * NT * D:L * NT * D + L * D]
        brp = bcast[:, L * NT * D + L * D:L * NT * D + 2 * L * D]
        fcb = bcast[:, L * NT * D + 2 * L * D:VX]

        epsc = sb.tile([128, 1], F32, name="epsc")
        nc.vector.memset(epsc[:], LN_EPS)

        # ---- h init + layer-0 gather table via AllGather(x) ----
        h_sb = sb.tile([128, NKC * D], F32, name="h_sb")
        nc.vector.tensor_copy(out=h_sb[:], in_=xh16[:])
        nc.gpsimd.dma_start(
            out=agin[0][:].rearrange("(k p) d -> p k d", p=128),
            in_=xh16[:].rearrange("p (k d) -> p k d", d=D))
        all_gather(0)

        aggr_sb = sb.tile([128, NKC * D], F32, name="aggr_sb")

        for l in range(L):
            w_l = w_sb[:, l * NCH:(l + 1) * NCH]
            C_l = cb_sb[:, (2 * l) * NKC:(2 * l + 1) * NKC]
            B_l = cb_sb[:, (2 * l + 1) * NKC:(2 * l + 2) * NKC]
            table = agout[l]

            # ------- gather + scatter (hw loop over windows, per pass) -----
            def cell_body(p, wv):
                # dynamic chunk offset for this (window, page) cell
                coff = ds(p * NW * KCu + wv * KCu, KCu)
                hsrc = ring3.tile([128, KCu * D], BF16, name="hsrc",
                                  tag="hsrc")
                nc.gpsimd.dma_gather(
                    out_ap=hsrc[:, :].rearrange("p (n d) -> p n d", d=D),
                    in_ap=table[p * PAGE:(p + 1) * PAGE, :],
                    idxs_ap=idx_sb[:, ds(p * NW * KCu * 8 + wv * (KCu * 8),
                                         KCu * 8)],
                    num_idxs=KCu * 128,
                    num_idxs_reg=KCu * 128,
                    elem_size=D,
                    single_packet=False)
                eqr = ring3.tile([128, KCu * 128], BF16, name="eqr",
                                 tag="eqr")
                eqv = eqr[:, :].rearrange("p (c t) -> p c t", t=128)
                nc.vector.tensor_tensor(
                    out=eqv,
                    in0=dcolb[:, coff, None].to_broadcast([128, KCu, 128]),
                    in1=iota[:, None, :].to_broadcast([128, KCu, 128]),
                    op=OP.is_equal)
                # scale one-hot by w_e in place (exact: rows are 0/1)
                nc.vector.tensor_tensor(
                    out=eqv, in0=eqv,
                    in1=w_l[:, coff][:, :, None].to_broadcast(
                        [128, KCu, 128]),
                    op=OP.mult)
                pmw = pM.tile([128, D], F32, name="pmw", tag="pmain",
                              bufs=2)
                for ci in range(KCu):
                    nc.tensor.matmul(
                        out=pmw[:, :],
                        lhsT=eqr[:, ci * 128:ci * 128 + 128],
                        rhs=hsrc[:, ci * D:(ci + 1) * D],
                        start=ci == 0, stop=ci == KCu - 1,
                        skip_group_check=True)
                ws = ts(wv, D)
                if p == 0:
                    nc.vector.tensor_copy(out=aggr_sb[:, ws], in_=pmw[:, :])
                else:
                    tcorr = ring3.tile([128, D], F32, name="tcorr",
                                       tag="tcorr")
                    tmul = ring3.tile([128, D], F32, name="tmul", tag="tmul")
                    nc.vector.tensor_tensor(
                        out=tcorr[:, :], in0=pmw[:, :],
                        in1=aggr_sb[:, ws], op=OP.add)
                    nc.vector.tensor_scalar(
                        tmul[:, :], h_sb[:, ws], C_l[:, ds(wv, 1)],
                        B_l[:, ds(wv, 1)], OP.mult, OP.subtract)
                    nc.vector.tensor_tensor(
                        out=aggr_sb[:, ws], in0=tcorr[:, :],
                        in1=tmul[:, :], op=OP.subtract)

            for p in range(2):
                with tc.For_i(0, NW, UN) as wb:
                    for u in range(UN):
                        cell_body(p, wb + u)

            # ------------- node phase (hw loop, 7x unrolled) -------------
            def node_body(kv):
                ks = ts(kv, D)
                astage = ring2.tile([128, D], BF16, name="astage",
                                    tag="astage")
                nc.vector.tensor_copy(out=astage[:, :], in_=aggr_sb[:, ks])
                aggT = ring2.tile([128, D], BF16, name="aggT", tag="aggT")
                nc.sync.dma_start_transpose(aggT[:, :], astage[:, :])
                pmlp = pM.tile([128, 2 * D], F32, name="pmlp", tag="pmlp",
                               bufs=2)
                for t in range(NT):
                    nwv = nwT_sb[:, (l * NT + t) * D:(l * NT + t + 1) * D]
                    nc.tensor.matmul(out=pmlp[:, t * D:(t + 1) * D],
                                     lhsT=aggT[:, :], rhs=nwv,
                                     start=True, stop=True,
                                     skip_group_check=True)
                ssel = ring3.tile([128, D], F32, name="ssel", tag="ssel")
                stmp = ring3.tile([128, D], F32, name="stmp", tag="stmp")
                nc.vector.tensor_tensor(
                    out=ssel[:, :], in0=pmlp[:, 0:D],
                    in1=nbr[:, (l * NT) * D:(l * NT + 1) * D], op=OP.add)
                nc.vector.tensor_tensor(
                    out=stmp[:, :], in0=pmlp[:, D:2 * D],
                    in1=nbr[:, (l * NT + 1) * D:(l * NT + 2) * D], op=OP.add)
                nc.vector.copy_predicated(
                    ssel[:, :], nm1[:, ds(kv, 1)].to_broadcast([128, D]),
                    stmp[:, :])
                hrelu = ring3.tile([128, D], F32, name="hrelu", tag="hrelu")
                sqscr = ring3.tile([128, D], F32, name="sqscr", tag="sqscr")
                musum = ring3.tile([128, 4], F32, name="musum", tag="musum")
                nc.scalar.activation(hrelu[:, :], ssel[:, :], AF.Relu,
                                     accum_out=musum[:, 0:1])
                nc.vector.tensor_scalar_mul(musum[:, 1:2], musum[:, 0:1],
                                            -1.0 / D)
                nc.scalar.activation(sqscr[:, :], hrelu[:, :], AF.Square,
                                     bias=musum[:, 1:2], scale=1.0,
                                     accum_out=musum[:, 2:3])
                nc.scalar.activation(musum[:, 3:4], musum[:, 2:3], AF.Sqrt,
                                     bias=epsc[:, 0:1], scale=1.0 / D)
                rstd = ring3.tile([128, 1], F32, name="rstd", tag="rstd")
                nc.vector.reciprocal(rstd[:, :], musum[:, 3:4])
                nc.vector.tensor_scalar(
                    stmp[:, :], hrelu[:, :], musum[:, 1:2], rstd[:, 0:1],
                    OP.add, OP.mult)
                nc.vector.tensor_tensor(
                    out=stmp[:, :], in0=stmp[:, :],
                    in1=grp[:, l * D:(l + 1) * D], op=OP.mult)
                nc.vector.tensor_tensor(
                    out=stmp[:, :], in0=stmp[:, :],
                    in1=brp[:, l * D:(l + 1) * D], op=OP.add)
                nc.vector.tensor_tensor(
                    out=h_sb[:, ks], in0=stmp[:, :], in1=h_sb[:, ks],
                    op=OP.add)

            with tc.For_i(0, NKC, UN) as kbase:
                for u in range(UN):
                    node_body(kbase + u)

            if l < L - 1:
                nc.gpsimd.dma_start(
                    out=agin[l + 1][:].rearrange("(k p) d -> p k d", p=128),
                    in_=h_sb[:].rearrange("p (k d) -> p k d", d=D))
                all_gather(l + 1)

        # ------------- final fc (hw loop, 7x unrolled) -------------
        def fc_body(kv):
            ks = ts(kv, D)
            hstage = ring2.tile([128, D], BF16, name="hstage", tag="astage")
            nc.vector.tensor_copy(out=hstage[:, :], in_=h_sb[:, ks])
            hT = ring2.tile([128, D], BF16, name="hT", tag="aggT")
            nc.sync.dma_start_transpose(hT[:, :], hstage[:, :])
            pfc = pM.tile([128, D], F32, name="pfc", tag="pmlp", bufs=2)
            nc.tensor.matmul(out=pfc[:, :], lhsT=hT[:, :], rhs=fcw_sb[:, :],
                             start=True, stop=True, skip_group_check=True)
            osb = ring2.tile([128, D], F16, name="osb", tag="osb")
            nc.vector.tensor_tensor(out=osb[:, :], in0=pfc[:, :],
                                    in1=fcb[:, :], op=OP.add)
            nc.sync.dma_start(out=t_out[ts(kv, 128), :], in_=osb[:, :])

        with tc.For_i(0, NKC, UN) as kbase:
            for u in range(UN):
                fc_body(kbase + u)

    nc.compile()
    return nc


# ---------------------------------------------------------------------------
_CACHE = {}


def kernel(**inputs):
    per_core, shared, meta = host_prep(**inputs)
    key = (meta['S'], meta['S0'], meta['S1'], meta['N'], meta['L'],
           meta['KCu'])
    if key not in _CACHE:
        _CACHE[key] = build_program(meta)
    nc = _CACHE[key]

    in_maps = []
    for c in range(CORES):
        pc = per_core[c]
        in_maps.append(dict(blob=per_core[c]['blob']))

    import os
    import time as _time
    trace = os.environ.get("KTRACE", "0") == "1"
    _t0 = _time.time()
    res = run_bass_kernel_spmd(nc, in_maps, core_ids=list(range(CORES)),
                               trace=trace)
    kernel.last_exec_wall = _time.time() - _t0
    R = meta['R']
    out = np.concatenate(
        [res.results[c]["out"][:R] for c in range(CORES)], axis=0)
    kernel.last_results = res
    return out.astype(np.float32)
